# revision 1
# baseline (speedup 1.0000x reference)
"""Trainium2 Bass kernel for one pre-LN transformer block (B=8, S=1024, H=768,
NH=12, I=3072), data-parallel over batch across 8 NeuronCores.

Contract: kernel(**inputs) takes the FULL unsharded inputs (as produced by
reference.setup_inputs()) and returns the FULL [8, 1024, 768] fp32 output.

Sharding: one batch element per core; weights replicated — no collectives.

Per-core design: natural layout = [token partitions, feature free]; T-layout =
[feature partitions, token free]. LayerNorm gains/biases are folded into the
downstream weights/biases on the host, so the on-chip LN is gain-free and its
bf16 output feeds a DMA-xbar transpose into T-layout. QKV produces q,k in
T-layout (weights stationary) and v in natural layout (activations
stationary), augmented with a ones column so the attention value-matmul also
accumulates the softmax denominator. Scores are computed transposed
[key, query] so exp() needs no cross-partition reductions (max-subtraction is
skipped; scores are a few units at most for this problem family). The
reciprocal denominator is applied as a per-partition DVE scale on PSUM
eviction. Head pairs are interleaved so their K=64 score matmuls land on
independent PE row tiles. All matmuls run in bf16 with fp32 PSUM
accumulation; LN stats, softmax and residuals stay fp32.
"""

import numpy as np
import ml_dtypes
from contextlib import ExitStack

import concourse.bass as bass
import concourse.mybir as mybir
import concourse.tile as tile
from concourse import bacc
from concourse.bass_utils import run_bass_kernel_spmd

B = 8
N_CORES = 8

P = 128
S, H, NH, HD, I = 1024, 768, 12, 64, 3072
SC = S // P      # 8 token chunks
KC = H // P      # 6 feature chunks
MC = I // P      # 24 fc1-output chunks
AF = mybir.ActivationFunctionType
BF16 = mybir.dt.bfloat16
F32 = mybir.dt.float32


def _build_block(nc, reps=1):
    x_d = nc.dram_tensor("x", [S, H], F32, kind="ExternalInput").ap()
    wqk_d = nc.dram_tensor("wqk", [H, 2 * H], BF16, kind="ExternalInput").ap()
    wv_d = nc.dram_tensor("wv", [H, H], BF16, kind="ExternalInput").ap()
    wp_d = nc.dram_tensor("wp", [H, H], BF16, kind="ExternalInput").ap()
    w1_d = nc.dram_tensor("w1", [H, I], BF16, kind="ExternalInput").ap()
    w2_d = nc.dram_tensor("w2", [I, H], BF16, kind="ExternalInput").ap()
    bqk_d = nc.dram_tensor("bqk", [P, 12], F32, kind="ExternalInput").ap()
    bv_d = nc.dram_tensor("bv", [1, H], F32, kind="ExternalInput").ap()
    bp_d = nc.dram_tensor("bp", [1, H], F32, kind="ExternalInput").ap()
    b1_d = nc.dram_tensor("b1", [P, MC], F32, kind="ExternalInput").ap()
    b2_d = nc.dram_tensor("b2", [1, H], F32, kind="ExternalInput").ap()
    out_d = nc.dram_tensor("out", [S, H], F32, kind="ExternalOutput").ap()

    with tile.TileContext(nc) as tc:
      for _rep in range(reps):
       with ExitStack() as top:
        const = top.enter_context(tc.tile_pool(name="const", bufs=1))

        eps_t = const.tile([P, 1], F32)
        nc.vector.memset(eps_t, 1e-5)

        bqk_sb = const.tile([P, 12], F32, name="bqk")
        b1_sb = const.tile([P, MC], F32, name="b1")
        bv_bc = const.tile([P, H], F32, name="bv")
        bp_bc = const.tile([P, H], F32, name="bp")
        b2_bc = const.tile([P, H], F32, name="b2")
        nc.sync.dma_start(out=bqk_sb, in_=bqk_d)
        nc.sync.dma_start(out=b1_sb, in_=b1_d)
        nc.gpsimd.dma_start(out=bv_bc, in_=bv_d.to_broadcast((P, H)))
        nc.gpsimd.dma_start(out=bp_bc, in_=bp_d.to_broadcast((P, H)))
        nc.gpsimd.dma_start(out=b2_bc, in_=b2_d.to_broadcast((P, H)))

        resid = top.enter_context(tc.tile_pool(name="resid", bufs=1))
        x_nat = resid.tile([P, SC, H], F32, name="x_nat")
        x1_nat = resid.tile([P, SC, H], F32, name="x1_nat")

        ln_tmp = top.enter_context(tc.tile_pool(name="ln_tmp", bufs=3))
        stat_tmp = top.enter_context(tc.tile_pool(name="stat_tmp", bufs=4))

        def layernorm_transpose(src_tile, dst_T, load_from=None):
            """src [P, SC, H] fp32 natural -> dst_T [P, KC, S] bf16 normalized
            (no gain/bias — folded into the consumer weights). The transpose
            runs on the DMA xbar."""
            for t in range(SC):
                if load_from is not None:
                    nc.sync.dma_start(
                        out=src_tile[:, t, :], in_=load_from[t * P:(t + 1) * P, :]
                    )
                xs = src_tile[:, t, :]
                stats = stat_tmp.tile([P, 3, 6], F32, name="bn_stats")
                for g in range(3):
                    nc.vector.bn_stats(
                        out=stats[:, g, :], in_=xs[:, g * 256:(g + 1) * 256]
                    )
                mv = stat_tmp.tile([P, 2], F32, name="bn_mv")
                nc.vector.bn_aggr(out=mv, in_=stats)
                sd = stat_tmp.tile([P, 1], F32, name="sd")
                nc.scalar.activation(
                    out=sd, in_=mv[:, 1:2], func=AF.Sqrt, bias=eps_t, scale=1.0
                )
                rstd = stat_tmp.tile([P, 1], F32, name="rstd")
                nc.vector.reciprocal(out=rstd, in_=sd)
                hpre = ln_tmp.tile([P, H], BF16, name="hpre")
                nc.vector.tensor_scalar(
                    out=hpre, in0=xs,
                    scalar1=mv[:, 0:1], scalar2=rstd,
                    op0=mybir.AluOpType.subtract, op1=mybir.AluOpType.mult,
                )
                nc.sync.dma_start_transpose(
                    out=dst_T[:, :, t * P:(t + 1) * P], in_=hpre
                )

        # ============ Phases 1-5: attention sublayer ============
        with ExitStack() as es_proj:
            attnT_pool = es_proj.enter_context(tc.tile_pool(name="attnT", bufs=1))
            attnT = attnT_pool.tile([P, KC, S], BF16, name="attnT")
            wp_pool = es_proj.enter_context(tc.tile_pool(name="wp", bufs=1))
            wp_sb = wp_pool.tile([P, KC, H], BF16, name="wp")

            with ExitStack() as es_an:
                attn_pool = es_an.enter_context(tc.tile_pool(name="attn_nat", bufs=1))
                attn_nat = attn_pool.tile([P, SC, H], BF16, name="attn_nat")

                with ExitStack() as es_a:
                    qkv_out = es_a.enter_context(tc.tile_pool(name="qkv_out", bufs=1))
                    qT = qkv_out.tile([P, KC, S], BF16, name="qT")
                    kT = qkv_out.tile([P, KC, S], BF16, name="kT")
                    v_aug = qkv_out.tile([P, SC, NH, HD + 1], BF16, name="v_aug")
                    nc.vector.memset(v_aug[:, :, :, HD:HD + 1], 1.0)

                    with ExitStack() as es_h:
                        hT_pool = es_h.enter_context(tc.tile_pool(name="hT", bufs=1))
                        hT = hT_pool.tile([P, KC, S], BF16, name="hT")
                        layernorm_transpose(x_nat, hT, load_from=x_d)

                        wpool = es_h.enter_context(tc.tile_pool(name="wqkv", bufs=1))
                        wqk_sb = wpool.tile([P, KC, 2 * H], BF16, name="wqk")
                        wv_sb = wpool.tile([P, KC, H], BF16, name="wv")
                        nc.sync.dma_start(
                            out=wqk_sb, in_=wqk_d.rearrange("(c p) n -> p c n", p=P)
                        )
                        nc.sync.dma_start(
                            out=wv_sb, in_=wv_d.rearrange("(c p) n -> p c n", p=P)
                        )
                        psum_mm = es_h.enter_context(
                            tc.tile_pool(name="psum_qkv", bufs=4, space="PSUM")
                        )
                        for m in range(12):
                            dst = qT if m < KC else kT
                            mc = m if m < KC else m - KC
                            for j in range(2):
                                ps = psum_mm.tile([P, 512], F32, name="mm")
                                for c in range(KC):
                                    nc.tensor.matmul(
                                        ps,
                                        lhsT=wqk_sb[:, c, m * P:(m + 1) * P],
                                        rhs=hT[:, c, j * 512:(j + 1) * 512],
                                        start=(c == 0), stop=(c == KC - 1),
                                    )
                                nc.vector.tensor_scalar_add(
                                    out=dst[:, mc, j * 512:(j + 1) * 512], in0=ps,
                                    scalar1=bqk_sb[:, m:m + 1],
                                )
                        for t in range(SC):
                            for j0, nsz in ((0, 512), (1, 256)):
                                ps = psum_mm.tile([P, 512], F32, name="mm")[:, :nsz]
                                for c in range(KC):
                                    nc.tensor.matmul(
                                        ps,
                                        lhsT=hT[:, c, t * P:(t + 1) * P],
                                        rhs=wv_sb[:, c, j0 * 512:j0 * 512 + nsz],
                                        start=(c == 0), stop=(c == KC - 1),
                                    )
                                hs, hn = j0 * 8, nsz // HD
                                nc.vector.tensor_tensor(
                                    out=v_aug[:, t, hs:hs + hn, 0:HD],
                                    in0=ps.rearrange("p (h d) -> p h d", d=HD),
                                    in1=bv_bc[:, j0 * 512:j0 * 512 + nsz].rearrange(
                                        "p (h d) -> p h d", d=HD
                                    ),
                                    op=mybir.AluOpType.add,
                                )

                    with ExitStack() as es_3:
                        expT_pool = es_3.enter_context(
                            tc.tile_pool(name="expT", bufs=4)
                        )
                        psum_sc = es_3.enter_context(
                            tc.tile_pool(name="psum_sc", bufs=6, space="PSUM")
                        )
                        psum_att = es_3.enter_context(
                            tc.tile_pool(name="psum_att", bufs=2, space="PSUM")
                        )
                        rec_pool = es_3.enter_context(tc.tile_pool(name="rec", bufs=4))
                        # Head pairs: even head operands live on partitions
                        # 0-63, odd head on 64-127, so their score matmuls land
                        # on independent PE row tiles (T0/T8). Software
                        # pipeline: emit pair p's scores+exp, then pair p-1's
                        # value matmuls, so the ACT exp lag of pair p hides
                        # behind PE attnV work of pair p-1.
                        NP2 = NH // 2
                        pair_expTs = {}

                        def scores_exp(hp):
                            ch = hp
                            expTs = [
                                expT_pool.tile([P, SC, S], BF16, name="expT")
                                for _ in range(2)
                            ]
                            pair_expTs[hp] = expTs
                            for i in range(SC):
                                for j in range(2):
                                    for e in range(2):
                                        po = e * HD
                                        ps = psum_sc.tile([P, 512], F32, name="sc")
                                        nc.tensor.matmul(
                                            ps,
                                            lhsT=kT[po:po + HD, ch,
                                                    i * P:(i + 1) * P],
                                            rhs=qT[po:po + HD, ch,
                                                   j * 512:(j + 1) * 512],
                                            start=True, stop=True,
                                        )
                                        nc.scalar.activation(
                                            out=expTs[e][:, i, j * 512:(j + 1) * 512],
                                            in_=ps, func=AF.Exp, scale=0.125,
                                        )

                        def attn_v(hp):
                            expTs = pair_expTs.pop(hp)
                            for t in range(SC):
                                for e in range(2):
                                    h = 2 * hp + e
                                    ps = psum_att.tile(
                                        [P, 512], F32, name="att"
                                    )[:, :HD + 1]
                                    for i in range(SC):
                                        nc.tensor.matmul(
                                            ps,
                                            lhsT=expTs[e][:, i, t * P:(t + 1) * P],
                                            rhs=v_aug[:, i, h, :],
                                            start=(i == 0), stop=(i == SC - 1),
                                        )
                                    rec = rec_pool.tile([P, 1], F32, name="rec")
                                    nc.vector.reciprocal(
                                        out=rec, in_=ps[:, HD:HD + 1]
                                    )
                                    nc.vector.tensor_scalar_mul(
                                        out=attn_nat[:, t, h * HD:(h + 1) * HD],
                                        in0=ps[:, 0:HD], scalar1=rec,
                                    )

                        for hp in range(NP2):
                            scores_exp(hp)
                            if hp > 0:
                                attn_v(hp - 1)
                        attn_v(NP2 - 1)

                # transpose attn (DMA xbar)
                for t in range(SC):
                    nc.sync.dma_start_transpose(
                        out=attnT[:, :, t * P:(t + 1) * P], in_=attn_nat[:, t, :]
                    )

            nc.sync.dma_start(out=wp_sb, in_=wp_d.rearrange("(c p) n -> p c n", p=P))
            with ExitStack() as es_5:
                psum_pj = es_5.enter_context(
                    tc.tile_pool(name="psum_pj", bufs=4, space="PSUM")
                )
                for t in range(SC):
                    for j0, nsz in ((0, 512), (1, 256)):
                        sl = slice(j0 * 512, j0 * 512 + nsz)
                        ps = psum_pj.tile([P, 512], F32, name="pj")[:, :nsz]
                        for c in range(KC):
                            nc.tensor.matmul(
                                ps,
                                lhsT=attnT[:, c, t * P:(t + 1) * P],
                                rhs=wp_sb[:, c, sl],
                                start=(c == 0), stop=(c == KC - 1),
                            )
                        nc.vector.tensor_tensor(
                            out=x1_nat[:, t, sl], in0=ps, in1=x_nat[:, t, sl],
                            op=mybir.AluOpType.add,
                        )
                        nc.vector.tensor_tensor(
                            out=x1_nat[:, t, sl], in0=x1_nat[:, t, sl],
                            in1=bp_bc[:, sl], op=mybir.AluOpType.add,
                        )

        # ============ Phases 6-8: MLP sublayer ============
        with ExitStack() as es_m1:
            m1_pool = es_m1.enter_context(tc.tile_pool(name="m1", bufs=1))
            m1T = m1_pool.tile([P, MC, S], BF16, name="m1T")

            with ExitStack() as es_f1:
                h2T_pool = es_f1.enter_context(tc.tile_pool(name="h2T", bufs=1))
                h2T = h2T_pool.tile([P, KC, S], BF16, name="h2T")
                layernorm_transpose(x1_nat, h2T)

                w1_pool = es_f1.enter_context(tc.tile_pool(name="w1", bufs=1))
                w1_sb = w1_pool.tile([P, KC, I], BF16, name="w1")
                nc.sync.dma_start(
                    out=w1_sb, in_=w1_d.rearrange("(c p) n -> p c n", p=P)
                )
                psum_f1 = es_f1.enter_context(
                    tc.tile_pool(name="psum_f1", bufs=4, space="PSUM")
                )
                for m in range(MC):
                    for j in range(2):
                        ps = psum_f1.tile([P, 512], F32, name="f1")
                        for c in range(KC):
                            nc.tensor.matmul(
                                ps,
                                lhsT=w1_sb[:, c, m * P:(m + 1) * P],
                                rhs=h2T[:, c, j * 512:(j + 1) * 512],
                                start=(c == 0), stop=(c == KC - 1),
                            )
                        nc.scalar.activation(
                            out=m1T[:, m, j * 512:(j + 1) * 512], in_=ps,
                            func=AF.Gelu_apprx_tanh, bias=b1_sb[:, m:m + 1],
                            scale=1.0,
                        )

            with ExitStack() as es_8:
                w2_pool = es_8.enter_context(tc.tile_pool(name="w2", bufs=1))
                w2_sb = w2_pool.tile([P, MC, H], BF16, name="w2")
                nc.sync.dma_start(
                    out=w2_sb, in_=w2_d.rearrange("(c p) n -> p c n", p=P)
                )
                out_pool = es_8.enter_context(tc.tile_pool(name="out_sb", bufs=3))
                psum_f2 = es_8.enter_context(
                    tc.tile_pool(name="psum_f2", bufs=4, space="PSUM")
                )
                for t in range(SC):
                    ot = out_pool.tile([P, H], F32, name="ot")
                    for j0, nsz in ((0, 512), (1, 256)):
                        sl = slice(j0 * 512, j0 * 512 + nsz)
                        ps = psum_f2.tile([P, 512], F32, name="f2")[:, :nsz]
                        for c in range(MC):
                            nc.tensor.matmul(
                                ps,
                                lhsT=m1T[:, c, t * P:(t + 1) * P],
                                rhs=w2_sb[:, c, sl],
                                start=(c == 0), stop=(c == MC - 1),
                            )
                        nc.vector.tensor_tensor(
                            out=ot[:, sl], in0=ps, in1=x1_nat[:, t, sl],
                            op=mybir.AluOpType.add,
                        )
                        nc.vector.tensor_tensor(
                            out=ot[:, sl], in0=ot[:, sl], in1=b2_bc[:, sl],
                            op=mybir.AluOpType.add,
                        )
                    nc.sync.dma_start(out=out_d[t * P:(t + 1) * P, :], in_=ot)

    return nc


def _host_prep(inputs):
    """Split/cast/reshape the full-model inputs into per-core DRAM tensors,
    folding the LayerNorm gains/biases into the downstream weights/biases.
    Returns (shared_map, per_core_x_list)."""
    import numpy as np
    import ml_dtypes

    bf16 = ml_dtypes.bfloat16
    f32 = np.float32
    qkv_w = np.asarray(inputs["qkv_w"], f32)
    qkv_b = np.asarray(inputs["qkv_b"], f32)
    fc1_w = np.asarray(inputs["fc1_w"], f32)
    fc1_b = np.asarray(inputs["fc1_b"], f32)
    g1 = np.asarray(inputs["ln1_g"], f32)
    bb1 = np.asarray(inputs["ln1_b"], f32)
    g2 = np.asarray(inputs["ln2_g"], f32)
    bb2 = np.asarray(inputs["ln2_b"], f32)

    wqkv_f = g1[:, None] * qkv_w            # gain folded into weights
    bqkv_f = bb1 @ qkv_w + qkv_b            # ln bias folded into bias
    w1_f = g2[:, None] * fc1_w
    b1_f = bb2 @ fc1_w + fc1_b

    shared = {
        "wqk": np.ascontiguousarray(wqkv_f[:, : 2 * H]).astype(bf16),
        "wv": np.ascontiguousarray(wqkv_f[:, 2 * H:]).astype(bf16),
        "wp": np.asarray(inputs["proj_w"], f32).astype(bf16),
        "w1": w1_f.astype(bf16),
        "w2": np.asarray(inputs["fc2_w"], f32).astype(bf16),
        "bqk": np.ascontiguousarray(bqkv_f[: 2 * H].reshape(12, P).T),
        "bv": np.ascontiguousarray(bqkv_f[2 * H:].reshape(1, H)),
        "bp": np.asarray(inputs["proj_b"], f32).reshape(1, H).copy(),
        "b1": np.ascontiguousarray(b1_f.reshape(MC, P).T),
        "b2": np.asarray(inputs["fc2_b"], f32).reshape(1, H).copy(),
    }
    x = np.asarray(inputs["x"], f32)
    xs = [np.ascontiguousarray(x[b]) for b in range(x.shape[0])]
    return shared, xs


_NC_CACHE = {}


def _get_nc(reps=1):
    if reps not in _NC_CACHE:
        nc = bacc.Bacc("TRN2", target_bir_lowering=False, debug=False,
                       num_devices=N_CORES)
        _build_block(nc, reps=reps)
        nc.compile()
        _NC_CACHE[reps] = nc
    return _NC_CACHE[reps]


def kernel(**inputs):
    nc = _get_nc()
    shared, xs = _host_prep(inputs)
    in_maps = [{**shared, "x": xs[c]} for c in range(N_CORES)]
    res = run_bass_kernel_spmd(nc, in_maps, list(range(N_CORES)))
    out = np.stack(
        [np.asarray(res.results[c]["out"], np.float32) for c in range(N_CORES)], 0
    )
    return out

